# revision 26
# baseline (speedup 1.0000x reference)
"""Mixtral-style MoE block (T=2048, H=1024, F=2048, E=8, top-2) on 8 trn2
NeuronCores.

Expert-parallel with host-side top-2 dispatch and F-split load balancing:
the (tiny) router runs on the host in fp32, exactly mirroring the reference
math.  Each expert's SwiGLU FFN is split along the hidden F dimension into
two halves living on two different cores, and each core carries one half of
a heavy expert (slot A) plus one half of a light expert (slot B) — so the
per-core work tracks the mean expert load, not the max.  Tokens ship in
bf16 once per slot; the cw-scaled copy for the (linear) w3 branch is built
on the idle vector engine from the tiny combine-weight row.  Each slot
computes out = (silu(x w1h) * (x_cw w3h)) @ w2h in bf16 with fp32 PSUM
accumulation and writes an [H, C] partial in bf16; the host scatter-adds
the four partials per token (2 experts x 2 F-halves).  No device
collectives; weights load once (bf16, 12.6 MB/core, hardware-DGE
contiguous chunks, fully SBUF-resident); a short junk-matmul warmup keeps
the PE busy while the DMA fabric delivers the first inputs so the HAM
clock-gate reaches 2.4 GHz before real work and never re-throttles.
"""
import numpy as np

try:
    import concourse  # noqa: F401
except ImportError:  # pragma: no cover
    import sys
    sys.path.insert(0, "/opt/trn_rl_repo")

import ml_dtypes
from concourse import mybir, bacc
import concourse.tile as tile
from concourse.bass_utils import run_bass_kernel_spmd

T, H, F, E, TOP_K = 2048, 1024, 2048, 8, 2
P = 128
F2 = F // 2    # F-half per slot
KH = H // P    # 8  k-tiles over H (mm1/mm3 contraction)
KF2 = F2 // P  # 8  k-tiles over F-half (mm2 contraction)
MF2 = F2 // P  # 8  m-tiles over F-half (mm1/mm3 output partitions)
MH = H // P    # 8  m-tiles over H (mm2 output partitions)
F32 = mybir.dt.float32
BF16 = mybir.dt.bfloat16
BF16NP = ml_dtypes.bfloat16
PSUM = "PSUM"

_NC_CACHE = {}


def _chunks(C):
    """Equal token-stream chunks of <=512 columns (PSUM bank limit)."""
    n = (C + 511) // 512
    base = C // n // 4 * 4
    cs, s = [], 0
    for i in range(n):
        e = C if i == n - 1 else s + base
        cs.append((s, e))
        s = e
    return cs


def build(CA, CB):
    nc = bacc.Bacc("TRN2", target_bir_lowering=False, debug=False,
                   num_devices=E)
    # host pre-tiles everything so each DMA is contiguous per partition
    xtA = nc.dram_tensor("xtA", [P, KH, CA], BF16, kind="ExternalInput")
    cwbA = nc.dram_tensor("cwbA", [P, CA], BF16, kind="ExternalInput")
    xtB = nc.dram_tensor("xtB", [P, KH, CB], BF16, kind="ExternalInput")
    cwbB = nc.dram_tensor("cwbB", [P, CB], BF16, kind="ExternalInput")
    w1A = nc.dram_tensor("w1A", [P, MF2, KH, P], BF16, kind="ExternalInput")
    w3A = nc.dram_tensor("w3A", [P, MF2, KH, P], BF16, kind="ExternalInput")
    w2A = nc.dram_tensor("w2A", [P, MH, KF2, P], BF16, kind="ExternalInput")
    w1B = nc.dram_tensor("w1B", [P, MF2, KH, P], BF16, kind="ExternalInput")
    w3B = nc.dram_tensor("w3B", [P, MF2, KH, P], BF16, kind="ExternalInput")
    w2B = nc.dram_tensor("w2B", [P, MH, KF2, P], BF16, kind="ExternalInput")
    outA = nc.dram_tensor("outA", [H, CA], BF16, kind="ExternalOutput")
    outB = nc.dram_tensor("outB", [H, CB], BF16, kind="ExternalOutput")

    CHA, CHB = _chunks(CA), _chunks(CB)
    pbA = 2 if len(CHA) <= 2 else 1
    pbB = 2 if len(CHB) <= 2 else 1
    WCH = [1, 1, 2, 4]  # 8 m-tiles of w1/w3 per slot, ramped chunk sizes
    with tile.TileContext(nc) as tc:
        with (
            tc.tile_pool(name="big", bufs=1) as big,
            tc.tile_pool(name="evac", bufs=4) as evac,
        ):
            # PE warmup so HAM un-throttles before real work arrives
            jt = big.tile([P, 512], BF16)
            nc.gpsimd.memset(jt[:], 0.0)
            with tc.tile_pool(name="warm", bufs=1, space=PSUM) as wps:
                jp = wps.tile([P, 512], F32)
                for _ in range(12):
                    nc.tensor.matmul(jp[:], lhsT=jt[:, :P], rhs=jt[:],
                                     start=True, stop=True)

            QK = KH // 4

            def token_tiles(tag, C):
                cwb_s = big.tile([P, C], BF16, name=f"cwb{tag}")
                xt_q = [big.tile([P, QK, C], BF16, name=f"xt{tag}{q}")
                        for q in range(4)]
                xts_s = big.tile([P, KH, C], BF16, name=f"xts{tag}")
                return cwb_s, xt_q, xts_s

            cwbA_s, xtA_q, xtsA_s = token_tiles("A", CA)
            cwbB_s, xtB_q, xtsB_s = token_tiles("B", CB)

            # slot A tokens: cwb + quarters 0-1 on the scalar ring,
            # quarters 2-3 woven into the sync ring's first weight chunks
            nc.scalar.dma_start(out=cwbA_s[:], in_=cwbA.ap())
            for q in range(2):
                nc.scalar.dma_start(out=xtA_q[q][:],
                                    in_=xtA.ap()[:, q * QK:(q + 1) * QK])

            def slot_weights(tag, w1d, w3d, xt_weave=None, xt_dram=None):
                """Interleaved w1/w3 chunk pairs so mm3's weights track
                mm1's; optionally weave token quarters 2-3 in early."""
                w1p, w3p = [None] * MF2, [None] * MF2
                m0 = 0
                for ci, g in enumerate(WCH):
                    t1 = big.tile([P, g, KH, P], BF16, name=f"w1{tag}c{m0}")
                    nc.sync.dma_start(out=t1[:], in_=w1d.ap()[:, m0:m0 + g])
                    if xt_weave is not None and ci < 2:
                        q = 2 + ci
                        nc.sync.dma_start(
                            out=xt_weave[q][:],
                            in_=xt_dram.ap()[:, q * QK:(q + 1) * QK])
                    t3 = big.tile([P, g, KH, P], BF16, name=f"w3{tag}c{m0}")
                    nc.sync.dma_start(out=t3[:], in_=w3d.ap()[:, m0:m0 + g])
                    for j in range(g):
                        w1p[m0 + j] = t1[:, j]
                        w3p[m0 + j] = t3[:, j]
                    m0 += g
                return w1p, w3p

            def w2_chunks(tag, dram):
                parts = [None] * MH
                for j in range(2):
                    g = MH // 2
                    t = big.tile([P, g, KF2, P], BF16, name=f"{tag}c{j}")
                    nc.sync.dma_start(out=t[:],
                                      in_=dram.ap()[:, j * g:(j + 1) * g])
                    for i in range(g):
                        parts[j * g + i] = t[:, i]
                return parts

            w1Ac, w3Ac = slot_weights("A", w1A, w3A, xtA_q, xtA)
            for k in range(KH):  # all slot-A token quarters issued by now
                nc.vector.tensor_tensor(xtsA_s[:, k], xtA_q[k // QK][:, k % QK],
                                        cwbA_s[:], op=mybir.AluOpType.mult)
            w2Ac = w2_chunks("w2A", w2A)
            # slot B tokens ride the scalar ring behind slot A's
            nc.scalar.dma_start(out=cwbB_s[:], in_=cwbB.ap())
            for q in range(4):
                nc.scalar.dma_start(out=xtB_q[q][:],
                                    in_=xtB.ap()[:, q * QK:(q + 1) * QK])
            for k in range(KH):
                nc.vector.tensor_tensor(xtsB_s[:, k], xtB_q[k // QK][:, k % QK],
                                        cwbB_s[:], op=mybir.AluOpType.mult)
            w1Bc, w3Bc = slot_weights("B", w1B, w3B)
            w2Bc = w2_chunks("w2B", w2B)

            def phase_a(tag, CH, pb, xt_q, xts_s, w1c, w3c, inter):
                with tc.tile_pool(name=f"psA{tag}", bufs=pb,
                                  space=PSUM) as psA:
                    for m in range(MF2):
                        ps1 = [psA.tile([P, e - s], F32, tag=f"ps1_{i}",
                                        name=f"ps1_{i}")
                               for i, (s, e) in enumerate(CH)]
                        ps3 = [psA.tile([P, e - s], F32, tag=f"ps3_{i}",
                                        name=f"ps3_{i}")
                               for i, (s, e) in enumerate(CH)]
                        for i, (s, e) in enumerate(CH):
                            for k in range(KH):
                                nc.tensor.matmul(
                                    ps1[i][:], lhsT=w1c[m][:, k, :],
                                    rhs=xt_q[k // QK][:, k % QK, s:e],
                                    start=(k == 0), stop=(k == KH - 1))
                        for i, (s, e) in enumerate(CH):
                            for k in range(KH):
                                nc.tensor.matmul(
                                    ps3[i][:], lhsT=w3c[m][:, k, :],
                                    rhs=xts_s[:, k, s:e],
                                    start=(k == 0), stop=(k == KH - 1))
                        for i, (s, e) in enumerate(CH):
                            sil = evac.tile([P, e - s], F32, tag=f"sil_{i}")
                            nc.scalar.activation(
                                sil[:], ps1[i][:],
                                mybir.ActivationFunctionType.Silu)
                            nc.vector.tensor_tensor(inter[:, m, s:e], sil[:],
                                                    ps3[i][:],
                                                    op=mybir.AluOpType.mult)

            def phase_b(tag, CH, pb, inter, w2c, outT, C):
                with tc.tile_pool(name=f"psB{tag}", bufs=2 * pb,
                                  space=PSUM) as psB:
                    for m in range(MH):
                        o = evac.tile([P, C], BF16, tag=f"o{tag}")
                        for i, (s, e) in enumerate(CH):
                            ps = psB.tile([P, e - s], F32, tag=f"ps_{i}",
                                          name=f"ps_{i}")
                            for k in range(KF2):
                                nc.tensor.matmul(
                                    ps[:], lhsT=w2c[m][:, k, :],
                                    rhs=inter[:, k, s:e],
                                    start=(k == 0), stop=(k == KF2 - 1))
                            nc.vector.tensor_copy(o[:, s:e], ps[:])
                            if m == MH - 1:  # final flush stays small
                                eng = nc.sync if i % 2 == 0 else nc.scalar
                                eng.dma_start(
                                    out=outT.ap()[m * P:(m + 1) * P, s:e],
                                    in_=o[:, s:e])
                        if m < MH - 1:
                            eng = nc.sync if m % 2 == 0 else nc.scalar
                            eng.dma_start(out=outT.ap()[m * P:(m + 1) * P, :],
                                          in_=o[:])

            interA = big.tile([P, KF2, CA], BF16, name="interA")
            interB = big.tile([P, KF2, CB], BF16, name="interB")
            phase_a("A", CHA, pbA, xtA_q, xtsA_s, w1Ac, w3Ac, interA)
            phase_b("A", CHA, pbA, interA, w2Ac, outA, CA)
            phase_a("B", CHB, pbB, xtB_q, xtsB_s, w1Bc, w3Bc, interB)
            phase_b("B", CHB, pbB, interB, w2Bc, outB, CB)
    nc.compile()
    return nc


def _route(hidden_states, gate_w):
    """Host router mirroring the reference fp32 math exactly."""
    logits = (hidden_states.astype(np.float32) @
              gate_w.astype(np.float32)).astype(np.float32)
    mx = logits.max(axis=-1, keepdims=True)
    p = np.exp(logits - mx)
    p /= p.sum(axis=-1, keepdims=True)
    idx = np.argsort(-p, axis=-1, kind="stable")[:, :TOP_K]
    tw = np.take_along_axis(p, idx, axis=-1)
    tw = tw / tw.sum(axis=-1, keepdims=True)
    return idx, tw


def _plan(hidden_states, gate_w):
    idx, tw = _route(hidden_states, gate_w)
    toks, cws = [], []
    for e in range(E):
        mask = idx == e
        tok = np.nonzero(mask.any(axis=1))[0]
        toks.append(tok)
        cws.append((tw * mask).sum(axis=1)[tok].astype(np.float32))
    counts = np.array([len(t) for t in toks])
    order = np.argsort(-counts, kind="stable")
    heavy, light = order[:4], order[4:]
    CA = (max(1, counts[heavy].max()) + 7) // 8 * 8
    CB = (max(1, counts[light].max()) + 7) // 8 * 8
    return toks, cws, heavy, light, int(CA), int(CB)


def _tok_inputs(x, tok, cw, C):
    n = len(tok)
    xe = x[tok]
    xt = np.zeros((P, KH, C), dtype=BF16NP)
    xt[:, :, :n] = xe.T.reshape(KH, P, n).transpose(1, 0, 2).astype(BF16NP)
    cwb = np.zeros((P, C), dtype=BF16NP)
    cwb[:, :n] = np.broadcast_to(cw.astype(BF16NP), (P, n))
    return xt, cwb


def _w_half(w1e, w3e, w2e, h):
    """bf16-tile the F-half h of one expert's weights."""
    w1h = w1e[:, h * F2:(h + 1) * F2]
    w3h = w3e[:, h * F2:(h + 1) * F2]
    w2h = w2e[h * F2:(h + 1) * F2, :]
    w1t = np.ascontiguousarray(
        w1h.reshape(KH, P, MF2, P).transpose(1, 2, 0, 3)).astype(BF16NP)
    w3t = np.ascontiguousarray(
        w3h.reshape(KH, P, MF2, P).transpose(1, 2, 0, 3)).astype(BF16NP)
    w2t = np.ascontiguousarray(
        w2h.reshape(KF2, P, MH, P).transpose(1, 2, 0, 3)).astype(BF16NP)
    return w1t, w3t, w2t


def make_in_maps(hidden_states, gate_w, w1, w2, w3):
    x = np.asarray(hidden_states, dtype=np.float32)
    toks, cws, heavy, light, CA, CB = _plan(
        x, np.asarray(gate_w, dtype=np.float32))
    in_maps = []
    for c in range(E):
        eA, eB, h = heavy[c // 2], light[c // 2], c % 2
        xtA, cwbA = _tok_inputs(x, toks[eA], cws[eA], CA)
        xtB, cwbB = _tok_inputs(x, toks[eB], cws[eB], CB)
        w1At, w3At, w2At = _w_half(np.asarray(w1[eA], dtype=np.float32),
                                   np.asarray(w3[eA], dtype=np.float32),
                                   np.asarray(w2[eA], dtype=np.float32), h)
        w1Bt, w3Bt, w2Bt = _w_half(np.asarray(w1[eB], dtype=np.float32),
                                   np.asarray(w3[eB], dtype=np.float32),
                                   np.asarray(w2[eB], dtype=np.float32), h)
        in_maps.append({"xtA": xtA, "cwbA": cwbA, "xtB": xtB, "cwbB": cwbB,
                        "w1A": w1At, "w3A": w3At, "w2A": w2At,
                        "w1B": w1Bt, "w3B": w3Bt, "w2B": w2Bt})
    return in_maps


def kernel(hidden_states, gate_w, w1, w2, w3):
    x = np.asarray(hidden_states, dtype=np.float32)
    gw = np.asarray(gate_w, dtype=np.float32)
    toks, cws, heavy, light, CA, CB = _plan(x, gw)
    if _NC_CACHE.get("C") != (CA, CB):
        _NC_CACHE["nc"] = build(CA, CB)
        _NC_CACHE["C"] = (CA, CB)
    nc = _NC_CACHE["nc"]

    in_maps = make_in_maps(x, gw, w1, w2, w3)
    res = run_bass_kernel_spmd(nc, in_maps, core_ids=list(range(E)),
                               trace=False)
    out = np.zeros((T, H), dtype=np.float32)
    for c in range(E):
        eA, eB = heavy[c // 2], light[c // 2]
        tA, tB = toks[eA], toks[eB]
        if len(tA):
            out[tA] += res.results[c]["outA"][:, :len(tA)]\
                .astype(np.float32).T
        if len(tB):
            out[tB] += res.results[c]["outB"][:, :len(tB)]\
                .astype(np.float32).T
    return out


# revision 27
# speedup vs baseline: 1.1898x; 1.1898x over previous
"""Mixtral-style MoE block (T=2048, H=1024, F=2048, E=8, top-2) on 8 trn2
NeuronCores.

Expert-parallel with host-side top-2 dispatch: the (tiny) router runs on the
host in fp32, exactly mirroring the reference math; each core receives only
the tokens routed to its expert (capacity C = max expert load), in bf16,
plus the renormalized top-2 combine weights; the cw-scaled token copy for
the (linear) w3 branch is built on the idle vector engine, so the combine
scaling costs no extra HBM traffic.  Each core computes
out_e = (silu(x w1) * (x_cw w3)) @ w2 in bf16 with fp32 PSUM accumulation
and writes its [H, C] partial in bf16; the host scatter-adds the two
partials per token.  No device collectives; weights load once (bf16,
12.6 MB/core, hardware-DGE contiguous chunks, fully SBUF-resident) and the
kernel is PE-bound at ~78 TF/s bf16 (~88 us of matmul streaming).  A short
junk-matmul warmup keeps the PE busy while the DMA fabric delivers the
first inputs, so the HAM clock-gate reaches 2.4 GHz before real work and
never re-throttles.
"""
import numpy as np

try:
    import concourse  # noqa: F401
except ImportError:  # pragma: no cover
    import sys
    sys.path.insert(0, "/opt/trn_rl_repo")

import ml_dtypes
from concourse import mybir, bacc
import concourse.tile as tile
from concourse.bass_utils import run_bass_kernel_spmd

T, H, F, E, TOP_K = 2048, 1024, 2048, 8, 2
P = 128
KH = H // P    # 8  k-tiles over H (mm1/mm3 contraction)
KF = F // P    # 16 k-tiles over F (mm2 contraction)
MF = F // P    # 16 m-tiles over F (mm1/mm3 output partitions)
MH = H // P    # 8  m-tiles over H (mm2 output partitions)
F32 = mybir.dt.float32
BF16 = mybir.dt.bfloat16
BF16NP = ml_dtypes.bfloat16
PSUM = "PSUM"

_NC_CACHE = {}


def _chunks(C):
    """Equal token-stream chunks of <=512 columns (PSUM bank limit)."""
    n = (C + 511) // 512
    base = C // n // 4 * 4
    cs, s = [], 0
    for i in range(n):
        e = C if i == n - 1 else s + base
        cs.append((s, e))
        s = e
    return cs


def build(C):
    nc = bacc.Bacc("TRN2", target_bir_lowering=False, debug=False,
                   num_devices=E)
    # host pre-tiles everything so each DMA is contiguous per partition
    xt = nc.dram_tensor("xt", [P, KH, C], BF16, kind="ExternalInput")
    cwb = nc.dram_tensor("cwb", [P, C], BF16, kind="ExternalInput")
    w1t = nc.dram_tensor("w1t", [P, MF, KH, P], BF16, kind="ExternalInput")
    w3t = nc.dram_tensor("w3t", [P, MF, KH, P], BF16, kind="ExternalInput")
    w2t = nc.dram_tensor("w2t", [P, MH, KF, P], BF16, kind="ExternalInput")
    outT = nc.dram_tensor("outT", [H, C], BF16, kind="ExternalOutput")

    CH = _chunks(C)
    psbufs_a = 2 if len(CH) <= 2 else 1
    psbufs_b = 4 if len(CH) <= 2 else 2
    with tile.TileContext(nc) as tc:
        with (
            tc.tile_pool(name="big", bufs=1) as big,
            tc.tile_pool(name="evac", bufs=4) as evac,
        ):
            # PE warmup so HAM un-throttles before real work arrives
            jt = big.tile([P, 512], BF16)
            nc.gpsimd.memset(jt[:], 0.0)
            with tc.tile_pool(name="warm", bufs=1, space=PSUM) as wps:
                jp = wps.tile([P, 512], F32)
                for _ in range(12):
                    nc.tensor.matmul(jp[:], lhsT=jt[:, :P], rhs=jt[:],
                                     start=True, stop=True)

            # combine weights (tiny) + token quarters 0-1 on the scalar
            # HWDGE ring; quarters 2-3 ride the sync ring between the first
            # weight chunks so all tokens land in the first DMA-lane wave.
            # The cw-scaled copy is built on the idle vector engine.
            QK = KH // 4
            cwb_s = big.tile([P, C], BF16)
            nc.scalar.dma_start(out=cwb_s[:], in_=cwb.ap())
            xt_q = [big.tile([P, QK, C], BF16, name=f"xt{q}")
                    for q in range(4)]
            for q in range(2):
                nc.scalar.dma_start(out=xt_q[q][:],
                                    in_=xt.ap()[:, q * QK:(q + 1) * QK])
            xts_s = big.tile([P, KH, C], BF16)

            def xtv(k):
                return xt_q[k // QK][:, k % QK]

            def xtsv(k):
                return xts_s[:, k]

            inter = big.tile([P, KF, C], BF16)

            # all weights SBUF-resident, contiguous DMAs on the sync
            # HWDGE ring, chunk sizes ramp up so the first m-tiles land fast
            WCH = [1, 1, 2, 4, 4, 4]
            w1c, w3c = [None] * MF, [None] * MF
            m0 = 0
            for ci, g in enumerate(WCH):
                t1 = big.tile([P, g, KH, P], BF16, name=f"w1c{m0}")
                nc.sync.dma_start(out=t1[:], in_=w1t.ap()[:, m0:m0 + g])
                if ci < 2:  # token quarters 2-3 between the first chunks
                    q = 2 + ci
                    nc.sync.dma_start(out=xt_q[q][:],
                                      in_=xt.ap()[:, q * QK:(q + 1) * QK])
                t3 = big.tile([P, g, KH, P], BF16, name=f"w3c{m0}")
                nc.sync.dma_start(out=t3[:], in_=w3t.ap()[:, m0:m0 + g])
                for j in range(g):
                    w1c[m0 + j] = t1[:, j]
                    w3c[m0 + j] = t3[:, j]
                m0 += g
                if ci == 1:  # all token quarters issued; build x_cw now
                    for k in range(KH):
                        nc.vector.tensor_tensor(xts_s[:, k],
                                                xt_q[k // QK][:, k % QK],
                                                cwb_s[:],
                                                op=mybir.AluOpType.mult)
            w2c = [None] * MH
            for j in range(2):
                g = MH // 2
                t = big.tile([P, g, KF, P], BF16, name=f"w2c{j}")
                nc.sync.dma_start(out=t[:], in_=w2t.ap()[:, j * g:(j + 1) * g])
                for i in range(g):
                    w2c[j * g + i] = t[:, i]

            # Phase A: interT[f, t] = silu(w1.T x) * (w3.T x_cw)
            with tc.tile_pool(name="psA", bufs=psbufs_a, space=PSUM) as psA:
                for m in range(MF):
                    ps1 = [psA.tile([P, e - s], F32, tag=f"ps1_{i}",
                                    name=f"ps1_{i}")
                           for i, (s, e) in enumerate(CH)]
                    ps3 = [psA.tile([P, e - s], F32, tag=f"ps3_{i}",
                                    name=f"ps3_{i}")
                           for i, (s, e) in enumerate(CH)]
                    for i, (s, e) in enumerate(CH):
                        for k in range(KH):
                            nc.tensor.matmul(ps1[i][:], lhsT=w1c[m][:, k, :],
                                             rhs=xtv(k)[:, s:e],
                                             start=(k == 0), stop=(k == KH - 1))
                    for i, (s, e) in enumerate(CH):
                        for k in range(KH):
                            nc.tensor.matmul(ps3[i][:], lhsT=w3c[m][:, k, :],
                                             rhs=xtsv(k)[:, s:e],
                                             start=(k == 0), stop=(k == KH - 1))
                    for i, (s, e) in enumerate(CH):
                        sil = evac.tile([P, e - s], F32, tag=f"sil_{i}")
                        nc.scalar.activation(sil[:], ps1[i][:],
                                             mybir.ActivationFunctionType.Silu)
                        nc.vector.tensor_tensor(inter[:, m, s:e], sil[:],
                                                ps3[i][:],
                                                op=mybir.AluOpType.mult)

            # Phase B: outT[h, t] = w2.T @ inter
            with tc.tile_pool(name="psB", bufs=psbufs_b, space=PSUM) as psB:
                for m in range(MH):
                    w2m = w2c[m]
                    o = evac.tile([P, C], BF16, tag="o")
                    for i, (s, e) in enumerate(CH):
                        ps = psB.tile([P, e - s], F32, tag=f"ps_{i}",
                                      name=f"ps_{i}")
                        for k in range(KF):
                            nc.tensor.matmul(ps[:], lhsT=w2m[:, k, :],
                                             rhs=inter[:, k, s:e],
                                             start=(k == 0), stop=(k == KF - 1))
                        nc.vector.tensor_copy(o[:, s:e], ps[:])
                        if m == MH - 1:  # last m-tile: fire per chunk so the
                            eng = nc.sync if i % 2 == 0 else nc.scalar
                            eng.dma_start(  # final flush is small
                                out=outT.ap()[m * P:(m + 1) * P, s:e],
                                in_=o[:, s:e])
                    if m < MH - 1:
                        eng = nc.sync if m % 2 == 0 else nc.scalar
                        eng.dma_start(out=outT.ap()[m * P:(m + 1) * P, :],
                                      in_=o[:])
    nc.compile()
    return nc


def _route(hidden_states, gate_w):
    """Host router mirroring the reference fp32 math exactly."""
    logits = (hidden_states.astype(np.float32) @
              gate_w.astype(np.float32)).astype(np.float32)
    mx = logits.max(axis=-1, keepdims=True)
    p = np.exp(logits - mx)
    p /= p.sum(axis=-1, keepdims=True)
    idx = np.argsort(-p, axis=-1, kind="stable")[:, :TOP_K]
    tw = np.take_along_axis(p, idx, axis=-1)
    tw = tw / tw.sum(axis=-1, keepdims=True)
    return idx, tw


def _plan(hidden_states, gate_w):
    idx, tw = _route(hidden_states, gate_w)
    toks, cws = [], []
    for e in range(E):
        mask = idx == e
        tok = np.nonzero(mask.any(axis=1))[0]
        toks.append(tok)
        cws.append((tw * mask).sum(axis=1)[tok].astype(np.float32))
    cap = max(1, max(len(t) for t in toks))
    C = (cap + 7) // 8 * 8
    return toks, cws, C


def make_in_maps(hidden_states, gate_w, w1, w2, w3):
    x = np.asarray(hidden_states, dtype=np.float32)
    toks, cws, C = _plan(x, np.asarray(gate_w, dtype=np.float32))
    in_maps = []
    for e in range(E):
        tok, cw = toks[e], cws[e]
        n = len(tok)
        xe = x[tok]                                   # [n, H] fp32
        xt = np.zeros((P, KH, C), dtype=BF16NP)
        xt[:, :, :n] = xe.T.reshape(KH, P, n).transpose(1, 0, 2).astype(BF16NP)
        cwb = np.zeros((P, C), dtype=BF16NP)
        cwb[:, :n] = np.broadcast_to(cw.astype(BF16NP), (P, n))
        w1e = np.asarray(w1[e], dtype=np.float32)
        w3e = np.asarray(w3[e], dtype=np.float32)
        w2e = np.asarray(w2[e], dtype=np.float32)
        # [H, F] -> [P, MF, KH, P] so any m-range DMA is contiguous
        w1tt = np.ascontiguousarray(
            w1e.reshape(KH, P, MF, P).transpose(1, 2, 0, 3)).astype(BF16NP)
        w3tt = np.ascontiguousarray(
            w3e.reshape(KH, P, MF, P).transpose(1, 2, 0, 3)).astype(BF16NP)
        w2tt = np.ascontiguousarray(
            w2e.reshape(KF, P, MH, P).transpose(1, 2, 0, 3)).astype(BF16NP)
        in_maps.append({"xt": xt, "cwb": cwb,
                        "w1t": w1tt, "w3t": w3tt, "w2t": w2tt})
    return in_maps


def kernel(hidden_states, gate_w, w1, w2, w3):
    x = np.asarray(hidden_states, dtype=np.float32)
    gw = np.asarray(gate_w, dtype=np.float32)
    toks, cws, C = _plan(x, gw)
    if _NC_CACHE.get("C") != C:
        _NC_CACHE["nc"] = build(C)
        _NC_CACHE["C"] = C
    nc = _NC_CACHE["nc"]

    in_maps = make_in_maps(x, gw, w1, w2, w3)
    res = run_bass_kernel_spmd(nc, in_maps, core_ids=list(range(E)),
                               trace=False)
    out = np.zeros((T, H), dtype=np.float32)
    for e in range(E):
        tok = toks[e]
        if len(tok):
            out[tok] += res.results[e]["outT"][:, :len(tok)]\
                .astype(np.float32).T
    return out

